# revision 1
# baseline (speedup 1.0000x reference)
"""Causal self-attention (B=4, S=2048, D=1024, H=16) on 8 Trainium2 NeuronCores.

Sharding: core c handles batch b = c // 2 and head-group g = c % 2
(8 heads, 512 of the 1024 output dims).  Data parallel over B, tensor
parallel over heads — attention is embarrassingly parallel over (b, h).

Per-core device program (identical on all cores, SPMD with different data):
  1. Projections: QT/KT in [d, q] layout (d on partitions), V in natural
     [k, d] layout with a ones-column appended (so the P@V matmul also
     produces the softmax denominator as an extra output row).
     All matmul operands fp16 (host-cast), accumulation in fp32 PSUM.
  2. Attention per head-pair: scoresT[k, q] tiles via row-packed (d=64)
     matmuls for two heads concurrently; exp on ScalarE with per-partition
     bias = -SHIFT + attention-mask bias (scale 1/sqrt(64) folded into Wq
     host-side); causal mask via tile skipping + one triangular 128x128
     multiply on diagonal tiles; PV accumulates ctxT[d(+1), q] over k-tiles.
  3. Unnormalized ctxT and the denominator row are DMA'd out; the host
     divides and re-assembles the [B, S, D] output.
"""

import numpy as np

B, S, D, H, HD = 4, 2048, 1024, 16, 64
DC = 512          # output dims per core (8 heads)
P = 128
NQC = S // 512    # q-chunks of 512
NKT = S // P      # k-tiles of 128
SHIFT = 8.0       # exp(score - SHIFT); cancels in the normalization
NEG = -30000.0    # attention-mask "minus infinity"

_PROG = None


def _emit_body(nc, t, pools):
    """One full compute pass: projections + attention + output DMA."""
    from concourse import mybir
    from concourse.bass import ds, ts

    f32 = mybir.dt.float32
    f16 = mybir.dt.float16
    EXP = mybir.ActivationFunctionType.Exp
    MULT = mybir.AluOpType.mult
    ADD = mybir.AluOpType.add
    epool, opool, psp, pss, psc = pools

    def proj_units(c):
        """Projection work for q-chunk c as a list of callables (one PSUM
        group each) so they can be interleaved into the attention stream."""
        qsl = ds(c * 512, 512)
        units = []

        def qk_unit(wt, bt, dst, dt):
            def run():
                pp = psp.tile([P, 2, 512], f32, tag="sc", name="pp")[:, 0, :] \
                    if psp is pss else psp.tile([P, 512], f32, tag="proj", name="pp")
                for s in range(8):
                    nc.tensor.matmul(
                        pp[:], wt[:, s, ts(dt, P)], t["ht"][:, s, qsl],
                        start=(s == 0), stop=(s == 7),
                    )
                nc.vector.tensor_scalar_add(
                    dst[:, dt, qsl], pp[:], bt[:, dt : dt + 1]
                )
            return run

        def v_unit(kt_i):
            def run():
                pp = psp.tile([P, 2, 512], f32, tag="sc", name="pp")[:, 0, :] \
                    if psp is pss else psp.tile([P, 512], f32, tag="proj", name="pp")
                for s in range(8):
                    nc.tensor.matmul(
                        pp[:], t["ht"][:, s, ds(kt_i * P, P)], t["wvt"][:, s, :],
                        start=(s == 0), stop=(s == 7),
                    )
                nc.vector.tensor_tensor(
                    t["v65"][:, kt_i, :, 0:64],
                    pp[:].rearrange("p (h d) -> p h d", h=8),
                    t["bvr_t"][:].rearrange("p (h d) -> p h d", h=8),
                    ADD,
                )
            return run

        for dt in range(4):
            units.append(qk_unit(t["wkt"], t["bk_t"], t["ktt"], dt))
            units.append(v_unit(4 * c + dt))
            units.append(qk_unit(t["wqt"], t["bq_t"], t["qt"], dt))
        return units

    def proj_steps(c):
        """Projection work for q-chunk c as fine-grained steps (2 matmuls
        each, ~430ns of PE) so they can fill PE slack inside the attention
        stream without ever delaying the scores->exp chain by much."""
        qsl = ds(c * 512, 512)
        steps = []

        def group(mm_args, fin):
            cell = {}

            def step(i):
                def run():
                    if i == 0:
                        cell["pp"] = psp.tile(
                            [P, 512], f32, tag="proj", name="pp"
                        )
                    pp = cell["pp"]
                    for s in (2 * i, 2 * i + 1):
                        lhsT, rhs = mm_args(s)
                        nc.tensor.matmul(
                            pp[:], lhsT, rhs, start=(s == 0), stop=(s == 7)
                        )
                    if i == 3:
                        fin(pp)
                return run

            steps.extend(step(i) for i in range(4))

        def qk_fin(dst, bt, dt):
            def fin(pp):
                nc.vector.tensor_scalar_add(
                    dst[:, dt, qsl], pp[:], bt[:, dt : dt + 1]
                )
            return fin

        def v_fin(kt_i):
            def fin(pp):
                nc.vector.tensor_tensor(
                    t["v65"][:, kt_i, :, 0:64],
                    pp[:].rearrange("p (h d) -> p h d", h=8),
                    t["bvr_t"][:].rearrange("p (h d) -> p h d", h=8),
                    ADD,
                )
            return fin

        for dt in range(4):
            wt = t["wkt"]
            group(
                lambda s, wt=wt, dt=dt: (wt[:, s, ts(dt, P)], t["ht"][:, s, qsl]),
                qk_fin(t["ktt"], t["bk_t"], dt),
            )
            kt_i = 4 * c + dt
            group(
                lambda s, kt_i=kt_i: (
                    t["ht"][:, s, ds(kt_i * P, P)], t["wvt"][:, s, :],
                ),
                v_fin(kt_i),
            )
            wq = t["wqt"]
            group(
                lambda s, wq=wq, dt=dt: (wq[:, s, ts(dt, P)], t["ht"][:, s, qsl]),
                qk_fin(t["qt"], t["bq_t"], dt),
            )
        return steps

    import os as _os
    order = _os.environ.get("K_ORDER", "fine")

    if order != "chunk":
        # prologue: emit chunk-0 projections in the order attention consumes
        # them — pair 0 needs K/Q of dt=0 and all four V tiles first, so
        # attention can start ~halfway through the prologue
        u0 = proj_units(0)  # [K0,V0,Q0, K1,V1,Q1, K2,V2,Q2, K3,V3,Q3]
        for i in (0, 2, 1, 4, 7, 10, 3, 5, 6, 8, 9, 11):
            u0[i]()
    queue = []
    for c in range(NQC):
        qsl = ds(c * 512, 512)
        if order == "chunk":
            # projections for this chunk right before its attention
            for u in proj_units(c):
                u()
            pending = []
        elif order == "ahead":
            # next chunk's projections before this chunk's attention
            pending = []
            if c + 1 < NQC:
                for u in proj_units(c + 1):
                    u()
        elif order == "fine":
            # fine-grained steps of next chunk's projections, popped one per
            # attention kt-iteration; leftovers drain at the chunk boundary
            pending = []
            queue = list(proj_steps(c + 1)) if c + 1 < NQC else []
        else:  # "ilv": next chunk's projections sprinkled per head-pair
            pending = proj_units(c + 1) if c + 1 < NQC else []
        # attention for q-chunk c, all 4 head-pairs
        nkt = 4 * c + 4
        for pr in range(4):
            for u in pending[3 * pr : 3 * pr + 3]:
                u()
            cA = psc.tile([P, 512], f32, tag="ctx", name="cA")
            cB = psc.tile([P, 512], f32, tag="ctx", name="cB")
            for kt_i in range(nkt):
                # causal: q columns [0, off) of this (k-tile, q-chunk) pair
                # are fully masked — skip them in scores, exp and PV.
                j = kt_i - 4 * c
                off = 128 * j if j > 0 else 0
                qso = ds(c * 512 + off, 512 - off)
                pt = pss.tile([P, 2, 512], f32, tag="sc")
                nc.tensor.matmul(
                    pt[:, 0, off:512],
                    t["ktt"][0:64, pr, ds(kt_i * P, P)],
                    t["qt"][0:64, pr, qso],
                    start=True, stop=True, tile_position=(0, 0),
                )
                nc.tensor.matmul(
                    pt[:, 1, off:512],
                    t["ktt"][64:128, pr, ds(kt_i * P, P)],
                    t["qt"][64:128, pr, qso],
                    start=True, stop=True, tile_position=(64, 0),
                )
                e = epool.tile([P, 2, 512], f16, tag="e")
                kbias = t["kb_t"][:, kt_i : kt_i + 1]
                nc.scalar.activation(
                    e[:, :, off:512], pt[:, :, off:512], EXP, bias=kbias
                )
                if j >= 0:
                    nc.vector.tensor_tensor(
                        e[:, :, off : off + P],
                        e[:, :, off : off + P],
                        t["tri_t"][:][:, None, :].to_broadcast((P, 2, P)),
                        MULT,
                    )
                nc.tensor.matmul(
                    cA[0:65, off:512],
                    t["v65"][:, kt_i, 2 * pr, :], e[:, 0, off:512],
                    start=(kt_i == 0), stop=(kt_i == nkt - 1),
                )
                nc.tensor.matmul(
                    cB[0:65, off:512],
                    t["v65"][:, kt_i, 2 * pr + 1, :], e[:, 1, off:512],
                    start=(kt_i == 0), stop=(kt_i == nkt - 1),
                )
                if queue:
                    queue.pop(0)()
            oA = opool.tile([P, 512], f32, tag="o", name="oA")
            oB = opool.tile([P, 512], f32, tag="o", name="oB")
            nc.vector.tensor_copy(oA[0:65, :], cA[0:65, :])
            nc.vector.tensor_copy(oB[0:65, :], cB[0:65, :])
            nc.sync.dma_start(t["out_d"][2 * pr, :, qsl], oA[0:65, :])
            nc.sync.dma_start(t["out_d"][2 * pr + 1, :, qsl], oB[0:65, :])
        while queue:  # finish next chunk's projections before its attention
            queue.pop(0)()


def _build(repeat=1):
    from contextlib import ExitStack

    import concourse.tile as tile
    from concourse import bacc, mybir

    f16, f32 = mybir.dt.float16, mybir.dt.float32

    nc = bacc.Bacc(
        "TRN2",
        target_bir_lowering=False,
        debug=False,
        enable_asserts=False,
        num_devices=8,
    )
    ht_d = nc.dram_tensor("ht", [D, S], f16, kind="ExternalInput").ap()
    wqt_d = nc.dram_tensor("wqt", [D, DC], f16, kind="ExternalInput").ap()
    wkt_d = nc.dram_tensor("wkt", [D, DC], f16, kind="ExternalInput").ap()
    wvt_d = nc.dram_tensor("wvt", [D, DC], f16, kind="ExternalInput").ap()
    bq_d = nc.dram_tensor("bq", [P, 4], f32, kind="ExternalInput").ap()
    bk_d = nc.dram_tensor("bk", [P, 4], f32, kind="ExternalInput").ap()
    bvr_d = nc.dram_tensor("bvr", [P, DC], f16, kind="ExternalInput").ap()
    kb_d = nc.dram_tensor("kbias", [P, NKT], f32, kind="ExternalInput").ap()
    tri_d = nc.dram_tensor("tri", [P, P], f16, kind="ExternalInput").ap()
    out_d = nc.dram_tensor("out", [8, 65, S], f32, kind="ExternalOutput").ap()

    import os as _os2

    with ExitStack() as ctx:
        tc = ctx.enter_context(tile.TileContext(nc))
        const = ctx.enter_context(tc.tile_pool(name="const", bufs=1))
        epool = ctx.enter_context(tc.tile_pool(name="epool", bufs=int(_os2.environ.get("K_EB", "8"))))
        opool = ctx.enter_context(tc.tile_pool(name="opool", bufs=4))
        import os as _os
        _sb = int(_os.environ.get("K_PSS_BUFS", "2"))
        _cb = int(_os.environ.get("K_PSC_BUFS", "2"))
        _pb = int(_os.environ.get("K_PSP_BUFS", "2"))
        pss = ctx.enter_context(tc.tile_pool(name="pss", bufs=_sb, space="PSUM"))
        if _pb:
            psp = ctx.enter_context(tc.tile_pool(name="psp", bufs=_pb, space="PSUM"))
        else:
            psp = pss  # projections share the scores pool slots
        psc = ctx.enter_context(tc.tile_pool(name="psc", bufs=_cb, space="PSUM"))

        t = dict(
            ht=const.tile([P, 8, S], f16, name="ht"),
            wqt=const.tile([P, 8, DC], f16, name="wqt"),
            wkt=const.tile([P, 8, DC], f16, name="wkt"),
            wvt=const.tile([P, 8, DC], f16, name="wvt"),
            qt=const.tile([P, 4, S], f16, name="qt"),
            ktt=const.tile([P, 4, S], f16, name="ktt"),
            v65=const.tile([P, NKT, 8, 65], f16, name="v65"),
            bq_t=const.tile([P, 4], f32, name="bq_t"),
            bk_t=const.tile([P, 4], f32, name="bk_t"),
            bvr_t=const.tile([P, DC], f16, name="bvr_t"),
            kb_t=const.tile([P, NKT], f32, name="kb_t"),
            tri_t=const.tile([P, P], f16, name="tri_t"),
            out_d=out_d,
        )

        nc.sync.dma_start(t["bq_t"][:], bq_d)
        nc.sync.dma_start(t["bk_t"][:], bk_d)
        nc.sync.dma_start(t["bvr_t"][:], bvr_d)
        nc.sync.dma_start(t["kb_t"][:], kb_d)
        nc.sync.dma_start(t["tri_t"][:], tri_d)
        # warmup exp so the ACT table load (~2.7us) hides behind startup DMAs
        warm = const.tile([P, 1], mybir.dt.float16, name="warm")
        nc.scalar.activation(
            warm[:], t["bq_t"][:, 0:1], mybir.ActivationFunctionType.Exp
        )
        # warm the PE (HAM clock gate) with dummy matmuls while the first
        # input DMAs are still in flight, so real work starts at 2.4 GHz
        if _os.environ.get("K_WARMPE", "1") == "1":
            scr = const.tile([P, 512], f16, name="scr")
            nc.vector.memset(scr[:], 0.0)
            wpt = (psp if psp is not pss else pss).tile(
                [P, 512], f32, tag="proj" if psp is not pss else "sc",
                name="wpt",
            )
            for i in range(14):
                nc.tensor.matmul(
                    wpt[:, 0:512], scr[:, 0:P], scr[:],
                    start=(i == 0), stop=(i == 13),
                )
        ht_r = ht_d.rearrange("(o p) m -> p o m", p=P)
        wq_r = wqt_d.rearrange("(o p) m -> p o m", p=P)
        wk_r = wkt_d.rearrange("(o p) m -> p o m", p=P)
        wv_r = wvt_d.rearrange("(o p) m -> p o m", p=P)
        # Load order: what q-chunk 0 needs first, so compute starts ASAP.
        for s in range(8):
            nc.sync.dma_start(t["wqt"][:, s, :], wq_r[:, s, :])
            nc.sync.dma_start(t["ht"][:, s, 0:512], ht_r[:, s, 0:512])
        for s in range(8):
            nc.sync.dma_start(t["wkt"][:, s, :], wk_r[:, s, :])
            nc.sync.dma_start(t["wvt"][:, s, :], wv_r[:, s, :])
        for s in range(8):
            nc.sync.dma_start(t["ht"][:, s, 512:2048], ht_r[:, s, 512:2048])
        nc.vector.memset(t["v65"][:, :, :, 64:65], 1.0)

        for _rep in range(repeat):
            _emit_body(nc, t, (epool, opool, psp, pss, psc))

    nc.compile()
    return nc


def _get_program():
    global _PROG
    if _PROG is None:
        _PROG = _build()
    return _PROG


def prepare_in_maps(hidden_states, attention_mask, Wq, bq, Wk, bk, Wv, bv):
    hidden_states = np.asarray(hidden_states, dtype=np.float32)
    attention_mask = np.asarray(attention_mask)
    Wq, bq = np.asarray(Wq, np.float32), np.asarray(bq, np.float32)
    Wk, bk = np.asarray(Wk, np.float32), np.asarray(bk, np.float32)
    Wv, bv = np.asarray(Wv, np.float32), np.asarray(bv, np.float32)
    tri = np.triu(np.ones((P, P), np.float16))  # tri[k, q] = 1 iff q >= k
    in_maps = []
    hts = [np.ascontiguousarray(hidden_states[b].T, dtype=np.float16)
           for b in range(B)]
    for c in range(8):
        b, g = divmod(c, 2)
        rows = slice(g * DC, (g + 1) * DC)
        am = np.asarray(attention_mask[b, 0, 0], np.float32)
        kbias = (np.where(am > 0, 0.0, NEG) - SHIFT).astype(np.float32)
        in_maps.append(
            dict(
                ht=hts[b],
                wqt=np.ascontiguousarray((Wq[rows] * 0.125).T, np.float16),
                wkt=np.ascontiguousarray(Wk[rows].T, np.float16),
                wvt=np.ascontiguousarray(Wv[rows].T, np.float16),
                bq=np.ascontiguousarray((bq[rows] * 0.125).reshape(4, P).T),
                bk=np.ascontiguousarray(bk[rows].reshape(4, P).T),
                bvr=np.broadcast_to(
                    bv[rows].astype(np.float16), (P, DC)
                ).copy(),
                kbias=np.ascontiguousarray(kbias.reshape(NKT, P).T),
                tri=tri,
            )
        )
    return in_maps


def _assemble(results):
    out = np.empty((B, S, D), np.float32)
    for c in range(8):
        b, g = divmod(c, 2)
        o = results[c]["out"]  # [8, 65, S] f32: rows 0..63 ctxT, row 64 denom
        ctx = o[:, :64, :] / o[:, 64:65, :]
        out[b, :, g * DC : (g + 1) * DC] = ctx.transpose(2, 0, 1).reshape(S, DC)
    return out


def _run(in_maps, trace=False):
    from concourse.bass_utils import run_bass_kernel_spmd

    nc = _get_program()
    return run_bass_kernel_spmd(nc, in_maps, core_ids=list(range(8)), trace=trace)


def kernel(hidden_states, attention_mask, Wq, bq, Wk, bk, Wv, bv):
    in_maps = prepare_in_maps(
        hidden_states, attention_mask, Wq, bq, Wk, bk, Wv, bv
    )
    res = _run(in_maps, trace=False)
    return _assemble(res.results)

